# revision 24
# baseline (speedup 1.0000x reference)
"""MemoryAugmentedLayer kernel for 8 trn2 NeuronCores.

Data-parallel over batch B=32768 (4096 rows/core); the two einsum partial
sums ([M,K] and [M,V]) are all-reduced between the write and read phases.

Structure (per core):
- Associativity: write logits = key_vec @ G with G = key_memory.T @ Wwr
  precomputed once (and read logits = qry_vec @ H with H = km_new.T @ Wrd
  computed once after the all-reduce). This removes the [B,M] score
  intermediates entirely and keeps every streaming matmul a 128-deep
  contraction with the weight stationary.
- Activations stay feature-major ([feat, batch]) on chip; PE transposes
  produce the batch-major copies the einsum and the output need.
- Precision: read path in float32r (~1e-4 rel err, full PE rate); write
  path (G, exp weights, einsum operands) in bf16 — it only perturbs the
  memory update, which is a ~5% correction to the memories.
"""

import numpy as np

import concourse.bacc as bacc
import concourse.mybir as mybir
import concourse.tile as tile
from concourse import masks
from concourse.bass_utils import run_bass_kernel_spmd

F32 = mybir.dt.float32
F32R = mybir.dt.float32r
BF16 = mybir.dt.bfloat16

B, D, M, K, V = 32768, 256, 1024, 128, 128
N_CORES = 8
B_LOC = B // N_CORES          # 4096 rows per core
CHUNK = 512                   # batch columns processed per chunk
NCH = B_LOC // CHUNK          # 8 chunks
NBT = CHUNK // 128            # 4 batch tiles of 128 per chunk
MT = M // 128                 # 8 tiles of the memory dim
DT = D // 128                 # 2 tiles of the input dim
INV_B = 1.0 / B


def build_nc(repeat=1):
    nc = bacc.Bacc("TRN2", target_bir_lowering=False, debug=False,
                   num_devices=N_CORES)

    x = nc.dram_tensor("x", [B_LOC, D], F32, kind="ExternalInput")
    Wk = nc.dram_tensor("Wk", [D, K], F32, kind="ExternalInput")
    Wv = nc.dram_tensor("Wv", [D, V], F32, kind="ExternalInput")
    Wq = nc.dram_tensor("Wq", [D, K], F32, kind="ExternalInput")
    bk = nc.dram_tensor("bk", [K, 1], F32, kind="ExternalInput")
    bv = nc.dram_tensor("bv", [V, 1], F32, kind="ExternalInput")
    bq = nc.dram_tensor("bq", [K, 1], F32, kind="ExternalInput")
    Wwr = nc.dram_tensor("Wwr", [M, M], F32, kind="ExternalInput")
    Wrd = nc.dram_tensor("Wrd", [M, M], F32, kind="ExternalInput")
    bwr = nc.dram_tensor("bwr", [M, 1], F32, kind="ExternalInput")
    brd = nc.dram_tensor("brd", [M, 1], F32, kind="ExternalInput")
    km = nc.dram_tensor("key_memory", [M, K], F32, kind="ExternalInput")
    vm = nc.dram_tensor("value_memory", [M, V], F32, kind="ExternalInput")
    y = nc.dram_tensor("y", [B_LOC, V], F32, kind="ExternalOutput")

    with tile.TileContext(nc) as tc:
        _emit(nc, tc, x, Wk, Wv, Wq, bk, bv, bq, Wwr, Wrd, bwr, brd, km, vm, y,
              repeat=repeat)
    nc.compile()
    return nc


def _emit(nc, tc, x, Wk, Wv, Wq, bk, bv, bq, Wwr, Wrd, bwr, brd, km, vm, y,
          repeat=1):
    AF = mybir.ActivationFunctionType
    ALU = mybir.AluOpType

    with (
        tc.tile_pool(name="resident", bufs=1) as rp,
        tc.tile_pool(name="stage", bufs=2) as stage,
        tc.tile_pool(name="stream", bufs=2) as sp,
        tc.tile_pool(name="stream1", bufs=1) as sp1,
        tc.tile_pool(name="ps_acc", bufs=1, space="PSUM") as ps_acc,
        tc.tile_pool(name="ps_mm", bufs=3, space="PSUM") as ps_mm,
        tc.tile_pool(name="ps_tr", bufs=1, space="PSUM") as ps_tr,
        tc.tile_pool(name="dram", bufs=1, space="DRAM") as dp,
    ):
        # ---------------- setup: identities, ones, biases ----------------
        ident = rp.tile([128, 128], F32)
        masks.make_identity(nc, ident[:])
        ident_b = rp.tile([128, 128], BF16)
        nc.vector.tensor_copy(ident_b[:], ident[:])

        ones_f = rp.tile([128, 1], F32)
        nc.gpsimd.memset(ones_f[:], 1.0)
        ones_r = rp.tile([128, 1], F32R)
        nc.vector.tensor_copy(ones_r[:], ones_f[:])
        one1 = rp.tile([1, 1], F32)
        nc.gpsimd.memset(one1[:], 1.0)

        # projection weights as lhsT ([d,128] blocks), rounded to f32r
        projw_r = rp.tile([128, DT, 3, 128], F32R)
        for j, W in enumerate((Wk, Wv, Wq)):
            for dt in range(DT):
                wst = stage.tile([128, 128], F32, tag="wst")
                nc.sync.dma_start(wst[:], W[dt * 128:(dt + 1) * 128, :])
                nc.vector.tensor_copy(projw_r[:, dt, j, :], wst[:])

        bias_p = rp.tile([128, 3], F32)
        for j, b in enumerate((bk, bv, bq)):
            nc.sync.dma_start(bias_p[:, j:j + 1], b[:])
        bias_pm1 = rp.tile([128, 3], F32)
        nc.vector.tensor_scalar_add(bias_pm1[:], bias_p[:], -1.0)
        bias_wr = rp.tile([128, MT], F32)
        bias_rd = rp.tile([128, MT], F32)
        for mp in range(MT):
            nc.sync.dma_start(bias_wr[:, mp:mp + 1], bwr[mp * 128:(mp + 1) * 128, :])
            nc.sync.dma_start(bias_rd[:, mp:mp + 1], brd[mp * 128:(mp + 1) * 128, :])

        # ---- G = key_memory.T @ Wwr (bf16), kmT/vmT (transposed, f32) ----
        kmT_f = rp.tile([128, M], F32)
        vmT_f = rp.tile([128, M], F32)
        g_lo = ps_acc.tile([128, 512], F32, tag="slot_a")
        g_hi = ps_acc.tile([128, 512], F32, tag="slot_b")
        for mk in range(MT):
            mst = stage.tile([128, 128], F32, tag="mst")
            nc.sync.dma_start(mst[:], km[mk * 128:(mk + 1) * 128, :])
            km_b = stage.tile([128, 128], BF16, tag="km_b")
            nc.vector.tensor_copy(km_b[:], mst[:])
            wwrt = stage.tile([128, M], F32, tag="wbig")
            nc.sync.dma_start(wwrt[:], Wwr[mk * 128:(mk + 1) * 128, :])
            wwrt_b = stage.tile([128, M], BF16, tag="wbig_b")
            nc.vector.tensor_copy(wwrt_b[:], wwrt[:])
            nc.tensor.matmul(g_lo[:], km_b[:], wwrt_b[:, 0:512],
                             start=(mk == 0), stop=(mk == MT - 1),
                             skip_group_check=True)
            nc.tensor.matmul(g_hi[:], km_b[:], wwrt_b[:, 512:M],
                             start=(mk == 0), stop=(mk == MT - 1),
                             skip_group_check=True)
            ptr = ps_tr.tile([128, 128], F32, tag="tr")
            nc.tensor.matmul(ptr[:], mst[:], ident[:], is_transpose=True,
                             start=True, stop=True)
            nc.scalar.copy(kmT_f[:, mk * 128:(mk + 1) * 128], ptr[:])
            mst2 = stage.tile([128, 128], F32, tag="mst")
            nc.sync.dma_start(mst2[:], vm[mk * 128:(mk + 1) * 128, :])
            ptr2 = ps_tr.tile([128, 128], F32, tag="tr")
            nc.tensor.matmul(ptr2[:], mst2[:], ident[:], is_transpose=True,
                             start=True, stop=True)
            nc.scalar.copy(vmT_f[:, mk * 128:(mk + 1) * 128], ptr2[:])
        G_b = rp.tile([128, M], BF16)
        nc.vector.tensor_copy(G_b[:, 0:512], g_lo[:])
        nc.vector.tensor_copy(G_b[:, 512:M], g_hi[:])

        # Wrd resident as lhsT tiles [128, M] f32r (read path)
        wrd_r = [rp.tile([128, M], F32R, name=f"wrd_r{i}") for i in range(MT)]
        for mk in range(MT):
            wst3 = stage.tile([128, M], F32, tag="wbig")
            nc.sync.dma_start(wst3[:], Wrd[mk * 128:(mk + 1) * 128, :])
            nc.vector.tensor_copy(wrd_r[mk][:], wst3[:])

        # qry kept for phase 2
        qryT_r = rp.tile([128, B_LOC], F32R)

        for _rep in range(repeat):
            _emit_rep(nc, tc, x, y, rp, sp, sp1, ps_acc, ps_mm, ps_tr, dp,
                      ident, ident_b, ones_r, one1, projw_r, bias_p, bias_pm1,
                      bias_wr, bias_rd, G_b, wrd_r, kmT_f, vmT_f, qryT_r)


def _emit_einsum(nc, sp, ps_mm, ps_tr, ident_b, carry, pk_lo, pk_hi, pv_acc,
                 first, last):
    ALU = mybir.AluOpType
    kvT, vvT, expw_bm, rw = carry
    kv_sc = sp.tile([128, NBT, 128], BF16, tag="kv_sc")
    vv_sc = sp.tile([128, NBT, 128], BF16, tag="vv_sc")
    for src, dstt in ((kvT, kv_sc), (vvT, vv_sc)):
        ptk = ps_tr.tile([128, NBT, 128], BF16, tag="trb", bufs=2)
        for t in range(NBT):
            nc.tensor.matmul(ptk[:, t, :], src[:, t * 128:(t + 1) * 128],
                             ident_b[:], is_transpose=True,
                             start=True, stop=True, skip_group_check=True)
        for t in range(NBT):
            nc.vector.tensor_scalar_mul(dstt[:, t, :], ptk[:, t, :],
                                        rw[:, t:t + 1])
    for t in range(NBT):
        f = first and t == 0
        l = last and t == NBT - 1
        nc.tensor.matmul(pk_lo[:], kv_sc[:, t, :], expw_bm[:, t, 0:512],
                         start=f, stop=l, skip_group_check=True)
        nc.tensor.matmul(pk_hi[:], kv_sc[:, t, :], expw_bm[:, t, 512:M],
                         start=f, stop=l, skip_group_check=True)
    for half in range(2):
        pvc = ps_mm.tile([128, 512], F32, tag="mm")
        for t in range(NBT):
            nc.tensor.matmul(pvc[:], vv_sc[:, t, :],
                             expw_bm[:, t, half * 512:(half + 1) * 512],
                             start=(t == 0), stop=(t == NBT - 1))
        dst = pv_acc[:, half * 512:(half + 1) * 512]
        if first:
            nc.vector.tensor_scalar_mul(dst, pvc[:], 1.0)
        else:
            nc.vector.scalar_tensor_tensor(dst, pvc[:], 1.0, dst,
                                           ALU.mult, ALU.add)


def _emit_rep(nc, tc, x, y, rp, sp, sp1, ps_acc, ps_mm, ps_tr, dp,
              ident, ident_b, ones_r, one1, projw_r, bias_p, bias_pm1,
              bias_wr, bias_rd, G_b, wrd_r, kmT_f, vmT_f, qryT_r):
    AF = mybir.ActivationFunctionType
    ALU = mybir.AluOpType
    AX = mybir.AxisListType

    # key einsum partials accumulate in PSUM across phase 1; value
    # partials accumulate in SBUF (per-chunk PSUM tile + DVE add) to keep
    # the PSUM bank budget at 8
    pk_lo = ps_acc.tile([128, 512], F32, tag="slot_a")
    pk_hi = ps_acc.tile([128, 512], F32, tag="slot_b")
    pv_acc = rp.tile([128, 2 * 512], F32, tag="pv_acc")

    # ======================= PHASE 1 =====================================
    x_tiled = x.rearrange("(h t p) d -> h p t d", p=128, t=NBT)
    carry = None  # (kvT, vvT, expw_bm, rw) of the previous chunk
    for h in range(NCH):
        # ---- load + transpose x chunk -> xTr [128, dtile, CHUNK] f32r
        xTr = sp.tile([128, DT, CHUNK], F32R, tag="xTr")
        xa = sp.tile([128, NBT, D], F32, tag="xa", bufs=3)
        nc.sync.dma_start(xa[:], x_tiled[h])
        xTr_v = xTr.rearrange("p dt (t2 s j) -> p t2 s dt j", s=2, j=128)
        for half in range(2):
            ptx = ps_tr.tile([128, 2, DT, 128], F32, tag="tr")
            for s in range(2):
                t = 2 * half + s
                for dt in range(DT):
                    nc.tensor.matmul(ptx[:, s, dt, :],
                                     xa[:, t, dt * 128:(dt + 1) * 128],
                                     ident[:], is_transpose=True,
                                     start=True, stop=True,
                                     skip_group_check=True)
            nc.any.tensor_copy(xTr_v[:, half], ptx[:])

        # ---- projections + elu -> kvT/vvT (bf16), qryT (f32r)
        kvT = sp.tile([128, CHUNK], BF16, tag="kvT")
        vvT = sp.tile([128, CHUNK], BF16, tag="vvT")
        for j in range(3):
            pp = ps_mm.tile([128, CHUNK], F32, tag="mm")
            for dt in range(DT):
                nc.tensor.matmul(pp[:], projw_r[:, dt, j, :], xTr[:, dt, :],
                                 start=(dt == 0), stop=(dt == DT - 1))
            # elu(z+b) = [max(z+b-1, -1)] + [min(exp(z+b), 1)]
            # (exp is monotone, so exp(min(w,0)) = min(exp(w), 1))
            texp = sp.tile([128, CHUNK], F32, tag="texp", bufs=2)
            nc.scalar.activation(texp[:], pp[:], AF.Exp,
                                 bias=bias_p[:, j:j + 1])
            trelu = sp.tile([128, CHUNK], F32, tag="trelu", bufs=2)
            nc.vector.tensor_scalar(out=trelu[:], in0=pp[:],
                                    scalar1=bias_pm1[:, j:j + 1],
                                    scalar2=-1.0, op0=ALU.add, op1=ALU.max)
            dst = (kvT[:], vvT[:],
                   qryT_r[:, h * CHUNK:(h + 1) * CHUNK])[j]
            nc.vector.scalar_tensor_tensor(dst, texp[:], 1.0, trelu[:],
                                           ALU.min, ALU.add)

        # ---- write logits (via G) + exp + batched transpose to batch-major
        expw_bm = sp1.tile([128, NBT, M], BF16, tag="expw_bm", bufs=2)
        for mp in range(MT):
            pwl = ps_mm.tile([128, CHUNK], F32, tag="mm")
            nc.tensor.matmul(pwl[:], G_b[:, mp * 128:(mp + 1) * 128], kvT[:],
                             start=True, stop=True)
            eT = sp.tile([128, CHUNK], BF16, tag="eT", bufs=2)
            nc.scalar.activation(eT[:], pwl[:], AF.Exp,
                                 bias=bias_wr[:, mp:mp + 1])
            ptb = ps_tr.tile([128, NBT, 128], BF16, tag="trb", bufs=2)
            for t in range(NBT):
                nc.tensor.matmul(ptb[:, t, :], eT[:, t * 128:(t + 1) * 128],
                                 ident_b[:], is_transpose=True,
                                 start=True, stop=True, skip_group_check=True)
            nc.any.tensor_copy(expw_bm[:, :, mp * 128:(mp + 1) * 128], ptb[:])

        # ---- softmax denominators (per batch row)
        rw = sp.tile([128, NBT], F32, tag="rw")
        sw = sp.tile([128, NBT], F32, tag="sw")
        for t in range(NBT):
            nc.vector.tensor_reduce(sw[:, t:t + 1], expw_bm[:, t, :],
                                    AX.X, ALU.add)
        nc.vector.reciprocal(rw[:], sw[:])

        # ---- einsum for the PREVIOUS chunk (skewed one chunk so PE has
        # this chunk's transposes/matmuls to chew on while DVE/ACT finish
        # the current chunk's exp/reduce chain)
        if carry is not None:
            _emit_einsum(nc, sp, ps_mm, ps_tr, ident_b, carry,
                         pk_lo, pk_hi, pv_acc,
                         first=(h == 1), last=False)
        carry = (kvT, vvT, expw_bm, rw)
    _emit_einsum(nc, sp, ps_mm, ps_tr, ident_b, carry,
                 pk_lo, pk_hi, pv_acc,
                 first=(NCH == 1), last=True)

    # ================== ALLREDUCE of partials ============================
    part_sb = rp.tile([128, M], F32, tag="part_sb")
    nc.vector.tensor_copy(part_sb[:, 0:512], pk_lo[:])
    nc.vector.tensor_copy(part_sb[:, 512:1024], pk_hi[:])
    cc_in = dp.tile([128, 2 * M], F32, tag="cc_in")
    cc_out = dp.tile([128, 2 * M], F32, tag="cc_out")
    nc.sync.dma_start(cc_in[:, 0:M], part_sb[:])
    nc.sync.dma_start(cc_in[:, M:2 * M], pv_acc[:])
    nc.gpsimd.collective_compute(
        "AllReduce", mybir.AluOpType.add,
        replica_groups=[list(range(N_CORES))],
        ins=[cc_in.opt()], outs=[cc_out.opt()],
    )
    red_sb = rp.tile([128, 2 * M], F32, tag="red_sb")
    nc.sync.dma_start(red_sb[:], cc_out[:])

    # ---- memory update + H = km_new.T @ Wrd (f32r) ----------------------
    km_newT = rp.tile([128, M], F32, tag="km_newT")
    nc.vector.scalar_tensor_tensor(km_newT[:], red_sb[:, 0:M], INV_B,
                                   kmT_f[:], ALU.mult, ALU.add)
    vm_newT = rp.tile([128, M], F32, tag="vm_newT")
    nc.vector.scalar_tensor_tensor(vm_newT[:], red_sb[:, M:2 * M], INV_B,
                                   vmT_f[:], ALU.mult, ALU.add)
    # transpose km_new/vm_new back to [m, *] blocks (f32r)
    kmn_mk = rp.tile([128, MT, 128], F32R, tag="kmn_mk")
    vmn_r = rp.tile([128, MT, 128], F32R, tag="vmn_r")
    for mk in range(MT):
        ptm = ps_tr.tile([128, 128], F32, tag="tr")
        nc.tensor.matmul(ptm[:], km_newT[:, mk * 128:(mk + 1) * 128],
                         ident[:], is_transpose=True, start=True, stop=True)
        nc.any.tensor_copy(kmn_mk[:, mk, :], ptm[:])
        ptm2 = ps_tr.tile([128, 128], F32, tag="tr")
        nc.tensor.matmul(ptm2[:], vm_newT[:, mk * 128:(mk + 1) * 128],
                         ident[:], is_transpose=True, start=True, stop=True)
        nc.any.tensor_copy(vmn_r[:, mk, :], ptm2[:])
    h_lo = ps_acc.tile([128, 512], F32, tag="slot_a")
    h_hi = ps_acc.tile([128, 512], F32, tag="slot_b")
    for mk in range(MT):
        nc.tensor.matmul(h_lo[:], kmn_mk[:, mk, :], wrd_r[mk][:, 0:512],
                         start=(mk == 0), stop=(mk == MT - 1),
                         skip_group_check=True)
        nc.tensor.matmul(h_hi[:], kmn_mk[:, mk, :], wrd_r[mk][:, 512:M],
                         start=(mk == 0), stop=(mk == MT - 1),
                         skip_group_check=True)
    H_r = rp.tile([128, M], F32R, tag="H_r")
    nc.vector.tensor_copy(H_r[:, 0:512], h_lo[:])
    nc.vector.tensor_copy(H_r[:, 512:M], h_hi[:])

    # ======================= PHASE 2 =====================================
    y_tiled = y.rearrange("(h t p) v -> h p t v", p=128, t=NBT)
    for h in range(NCH):
        qslice = qryT_r[:, h * CHUNK:(h + 1) * CHUNK]

        u_ps = ps_acc.tile([128, CHUNK], F32, tag="slot_a")
        s_ps = ps_acc.tile([1, CHUNK], F32, tag="slot_b")
        for mp in range(MT):
            prl = ps_mm.tile([128, CHUNK], F32, tag="mm")
            nc.tensor.matmul(prl[:], H_r[:, mp * 128:(mp + 1) * 128], qslice,
                             start=True, stop=True)
            erT = sp.tile([128, CHUNK], F32R, tag="erT", bufs=2)
            nc.scalar.activation(erT[:], prl[:], AF.Exp,
                                 bias=bias_rd[:, mp:mp + 1])
            nc.tensor.matmul(u_ps[:], vmn_r[:, mp, :], erT[:],
                             start=(mp == 0), stop=(mp == MT - 1),
                             skip_group_check=True)
            nc.tensor.matmul(s_ps[:], ones_r[:], erT[:],
                             start=(mp == 0), stop=(mp == MT - 1),
                             skip_group_check=True)

        # transpose denominators [1, CHUNK] -> [128, NBT] and invert
        s_sb = sp.tile([1, CHUNK], F32, tag="s_sb")
        nc.any.tensor_copy(s_sb[:], s_ps[:])
        s_cols = sp.tile([128, NBT], F32, tag="s_cols")
        for t in range(NBT):
            pst = ps_mm.tile([128, 1], F32, tag="mm")
            nc.tensor.matmul(pst[:], s_sb[0:1, t * 128:(t + 1) * 128],
                             one1[:], start=True, stop=True)
            nc.vector.tensor_copy(s_cols[:, t:t + 1], pst[:])
        r_cols = sp.tile([128, NBT], F32, tag="r_cols")
        nc.vector.reciprocal(r_cols[:], s_cols[:])

        # read_vec: transpose u back to batch-major, scale, store
        u_sb = sp.tile([128, CHUNK], F32, tag="u_sb")
        nc.any.tensor_copy(u_sb[:], u_ps[:])
        ot = sp.tile([128, NBT, V], F32, tag="ot", bufs=2)
        for t in range(NBT):
            ptu = ps_tr.tile([128, 128], F32, tag="tr")
            nc.tensor.matmul(ptu[:], u_sb[:, t * 128:(t + 1) * 128],
                             ident[:], is_transpose=True,
                             start=True, stop=True)
            nc.vector.tensor_scalar_mul(ot[:, t, :], ptu[:],
                                        r_cols[:, t:t + 1])
        nc.sync.dma_start(y_tiled[h], ot[:])


_NC_CACHE = None


def _get_nc():
    global _NC_CACHE
    if _NC_CACHE is None:
        _NC_CACHE = build_nc()
    return _NC_CACHE


def kernel(**inputs):
    nc = _get_nc()
    xs = np.ascontiguousarray(np.asarray(inputs["x"], dtype=np.float32))
    rep = {}
    for name in ("Wk", "Wv", "Wq", "Wwr", "Wrd", "key_memory", "value_memory"):
        rep[name] = np.ascontiguousarray(np.asarray(inputs[name], np.float32))
    for name in ("bk", "bv", "bq", "bwr", "brd"):
        rep[name] = np.ascontiguousarray(
            np.asarray(inputs[name], np.float32).reshape(-1, 1))
    in_maps = []
    for c in range(N_CORES):
        m = {"x": xs[c * B_LOC:(c + 1) * B_LOC]}
        m.update(rep)
        in_maps.append(m)
    res = run_bass_kernel_spmd(nc, in_maps, core_ids=list(range(N_CORES)))
    return np.concatenate([r["y"] for r in res.results], axis=0)


# revision 25
# speedup vs baseline: 1.2649x; 1.2649x over previous
"""MemoryAugmentedLayer kernel for 8 trn2 NeuronCores.

Data-parallel over batch B=32768 (4096 rows/core); the two einsum partial
sums ([M,K] and [M,V]) are all-reduced between the write and read phases.

Structure (per core):
- Associativity: write logits = key_vec @ G with G = key_memory.T @ Wwr
  precomputed once (and read logits = qry_vec @ H with H = km_new.T @ Wrd
  computed once after the all-reduce). This removes the [B,M] score
  intermediates entirely and keeps every streaming matmul a 128-deep
  contraction with the weight stationary.
- Activations stay feature-major ([feat, batch]) on chip; PE transposes
  produce the batch-major copies the einsum and the output need.
- Precision: read path in float32r (~1e-4 rel err, full PE rate); write
  path (G, exp weights, einsum operands) in bf16 — it only perturbs the
  memory update, which is a ~5% correction to the memories.
"""

import numpy as np

import concourse.bacc as bacc
import concourse.mybir as mybir
import concourse.tile as tile
from concourse import masks
from concourse.bass_utils import run_bass_kernel_spmd

F32 = mybir.dt.float32
F32R = mybir.dt.float32r
BF16 = mybir.dt.bfloat16

B, D, M, K, V = 32768, 256, 1024, 128, 128
N_CORES = 8
B_LOC = B // N_CORES          # 4096 rows per core
CHUNK = 512                   # batch columns processed per chunk
NCH = B_LOC // CHUNK          # 8 chunks
NBT = CHUNK // 128            # 4 batch tiles of 128 per chunk
MT = M // 128                 # 8 tiles of the memory dim
DT = D // 128                 # 2 tiles of the input dim
INV_B = 1.0 / B


def build_nc(repeat=1):
    nc = bacc.Bacc("TRN2", target_bir_lowering=False, debug=False,
                   num_devices=N_CORES)

    x = nc.dram_tensor("x", [B_LOC, D], F32, kind="ExternalInput")
    Wk = nc.dram_tensor("Wk", [D, K], F32, kind="ExternalInput")
    Wv = nc.dram_tensor("Wv", [D, V], F32, kind="ExternalInput")
    Wq = nc.dram_tensor("Wq", [D, K], F32, kind="ExternalInput")
    bk = nc.dram_tensor("bk", [K, 1], F32, kind="ExternalInput")
    bv = nc.dram_tensor("bv", [V, 1], F32, kind="ExternalInput")
    bq = nc.dram_tensor("bq", [K, 1], F32, kind="ExternalInput")
    Wwr = nc.dram_tensor("Wwr", [M, M], F32, kind="ExternalInput")
    Wrd = nc.dram_tensor("Wrd", [M, M], F32, kind="ExternalInput")
    bwr = nc.dram_tensor("bwr", [M, 1], F32, kind="ExternalInput")
    brd = nc.dram_tensor("brd", [M, 1], F32, kind="ExternalInput")
    km = nc.dram_tensor("key_memory", [M, K], F32, kind="ExternalInput")
    vm = nc.dram_tensor("value_memory", [M, V], F32, kind="ExternalInput")
    y = nc.dram_tensor("y", [B_LOC, V], F32, kind="ExternalOutput")

    with tile.TileContext(nc) as tc:
        _emit(nc, tc, x, Wk, Wv, Wq, bk, bv, bq, Wwr, Wrd, bwr, brd, km, vm, y,
              repeat=repeat)
    nc.compile()
    return nc


def _emit(nc, tc, x, Wk, Wv, Wq, bk, bv, bq, Wwr, Wrd, bwr, brd, km, vm, y,
          repeat=1):
    AF = mybir.ActivationFunctionType
    ALU = mybir.AluOpType

    with (
        tc.tile_pool(name="resident", bufs=1) as rp,
        tc.tile_pool(name="stage", bufs=2) as stage,
        tc.tile_pool(name="stream", bufs=2) as sp,
        tc.tile_pool(name="stream1", bufs=1) as sp1,
        tc.tile_pool(name="ps_acc", bufs=1, space="PSUM") as ps_acc,
        tc.tile_pool(name="ps_mm", bufs=3, space="PSUM") as ps_mm,
        tc.tile_pool(name="ps_tr", bufs=1, space="PSUM") as ps_tr,
        tc.tile_pool(name="dram", bufs=1, space="DRAM") as dp,
    ):
        # ---------------- setup: identities, ones, biases ----------------
        ident = rp.tile([128, 128], F32)
        masks.make_identity(nc, ident[:])
        ident_b = rp.tile([128, 128], BF16)
        nc.vector.tensor_copy(ident_b[:], ident[:])

        ones_f = rp.tile([128, 1], F32)
        nc.gpsimd.memset(ones_f[:], 1.0)
        ones_r = rp.tile([128, 1], F32R)
        nc.vector.tensor_copy(ones_r[:], ones_f[:])
        one1 = rp.tile([1, 1], F32)
        nc.gpsimd.memset(one1[:], 1.0)

        # projection weights as lhsT ([d,128] blocks), rounded to f32r
        projw_r = rp.tile([128, DT, 3, 128], F32R)
        for j, W in enumerate((Wk, Wv, Wq)):
            for dt in range(DT):
                wst = stage.tile([128, 128], F32, tag="wst")
                nc.sync.dma_start(wst[:], W[dt * 128:(dt + 1) * 128, :])
                nc.vector.tensor_copy(projw_r[:, dt, j, :], wst[:])

        bias_p = rp.tile([128, 3], F32)
        for j, b in enumerate((bk, bv, bq)):
            nc.sync.dma_start(bias_p[:, j:j + 1], b[:])
        bias_pm1 = rp.tile([128, 3], F32)
        nc.vector.tensor_scalar_add(bias_pm1[:], bias_p[:], -1.0)
        bias_wr = rp.tile([128, MT], F32)
        bias_rd = rp.tile([128, MT], F32)
        for mp in range(MT):
            nc.sync.dma_start(bias_wr[:, mp:mp + 1], bwr[mp * 128:(mp + 1) * 128, :])
            nc.sync.dma_start(bias_rd[:, mp:mp + 1], brd[mp * 128:(mp + 1) * 128, :])

        # ---- G = key_memory.T @ Wwr (bf16), kmT/vmT (transposed, f32) ----
        kmT_f = rp.tile([128, M], F32)
        vmT_f = rp.tile([128, M], F32)
        g_lo = ps_acc.tile([128, 512], F32, tag="slot_a")
        g_hi = ps_acc.tile([128, 512], F32, tag="slot_b")
        for mk in range(MT):
            mst = stage.tile([128, 128], F32, tag="mst")
            nc.sync.dma_start(mst[:], km[mk * 128:(mk + 1) * 128, :])
            km_b = stage.tile([128, 128], BF16, tag="km_b")
            nc.vector.tensor_copy(km_b[:], mst[:])
            wwrt = stage.tile([128, M], F32, tag="wbig")
            nc.sync.dma_start(wwrt[:], Wwr[mk * 128:(mk + 1) * 128, :])
            wwrt_b = stage.tile([128, M], BF16, tag="wbig_b")
            nc.vector.tensor_copy(wwrt_b[:], wwrt[:])
            nc.tensor.matmul(g_lo[:], km_b[:], wwrt_b[:, 0:512],
                             start=(mk == 0), stop=(mk == MT - 1),
                             skip_group_check=True)
            nc.tensor.matmul(g_hi[:], km_b[:], wwrt_b[:, 512:M],
                             start=(mk == 0), stop=(mk == MT - 1),
                             skip_group_check=True)
            ptr = ps_tr.tile([128, 128], F32, tag="tr")
            nc.tensor.matmul(ptr[:], mst[:], ident[:], is_transpose=True,
                             start=True, stop=True)
            nc.scalar.copy(kmT_f[:, mk * 128:(mk + 1) * 128], ptr[:])
            mst2 = stage.tile([128, 128], F32, tag="mst")
            nc.sync.dma_start(mst2[:], vm[mk * 128:(mk + 1) * 128, :])
            ptr2 = ps_tr.tile([128, 128], F32, tag="tr")
            nc.tensor.matmul(ptr2[:], mst2[:], ident[:], is_transpose=True,
                             start=True, stop=True)
            nc.scalar.copy(vmT_f[:, mk * 128:(mk + 1) * 128], ptr2[:])
        G_b = rp.tile([128, M], BF16)
        nc.vector.tensor_copy(G_b[:, 0:512], g_lo[:])
        nc.vector.tensor_copy(G_b[:, 512:M], g_hi[:])

        # Wrd resident as lhsT tiles [128, M] f32r (read path); the DMAs
        # are emitted lazily (inside the first rep, before H) so they don't
        # compete with phase-1's x streaming for DMA queues at startup
        wrd_r = [rp.tile([128, M], F32R, name=f"wrd_r{i}") for i in range(MT)]
        wrd_loaded = [False]

        def load_wrd():
            if wrd_loaded[0]:
                return
            wrd_loaded[0] = True
            for mk in range(MT):
                wst3 = stage.tile([128, M], F32, tag="wbig")
                nc.sync.dma_start(wst3[:], Wrd[mk * 128:(mk + 1) * 128, :])
                nc.vector.tensor_copy(wrd_r[mk][:], wst3[:])

        # qry kept for phase 2
        qryT_r = rp.tile([128, B_LOC], F32R)

        for _rep in range(repeat):
            _emit_rep(nc, tc, x, y, rp, sp, sp1, ps_acc, ps_mm, ps_tr, dp,
                      ident, ident_b, ones_r, one1, projw_r, bias_p, bias_pm1,
                      bias_wr, bias_rd, G_b, wrd_r, kmT_f, vmT_f, qryT_r,
                      load_wrd)


def _emit_einsum(nc, sp, ps_mm, ps_tr, ident_b, carry, pk_lo, pk_hi, pv_acc,
                 first, last):
    ALU = mybir.AluOpType
    kvT, vvT, expw_bm, rw = carry
    kv_sc = sp.tile([128, NBT, 128], BF16, tag="kv_sc")
    vv_sc = sp.tile([128, NBT, 128], BF16, tag="vv_sc")
    for src, dstt in ((kvT, kv_sc), (vvT, vv_sc)):
        ptk = ps_tr.tile([128, NBT, 128], BF16, tag="trb", bufs=2)
        for t in range(NBT):
            nc.tensor.matmul(ptk[:, t, :], src[:, t * 128:(t + 1) * 128],
                             ident_b[:], is_transpose=True,
                             start=True, stop=True, skip_group_check=True)
        for t in range(NBT):
            nc.vector.tensor_scalar_mul(dstt[:, t, :], ptk[:, t, :],
                                        rw[:, t:t + 1])
    for t in range(NBT):
        f = first and t == 0
        l = last and t == NBT - 1
        nc.tensor.matmul(pk_lo[:], kv_sc[:, t, :], expw_bm[:, 0:4, t, :],
                         start=f, stop=l, skip_group_check=True)
        nc.tensor.matmul(pk_hi[:], kv_sc[:, t, :], expw_bm[:, 4:8, t, :],
                         start=f, stop=l, skip_group_check=True)
    for half in range(2):
        pvc = ps_mm.tile([128, 512], F32, tag="mm")
        for t in range(NBT):
            nc.tensor.matmul(pvc[:], vv_sc[:, t, :],
                             expw_bm[:, half * 4:(half + 1) * 4, t, :],
                             start=(t == 0), stop=(t == NBT - 1))
        dst = pv_acc[:, half * 512:(half + 1) * 512]
        if first:
            nc.vector.tensor_scalar_mul(dst, pvc[:], 1.0)
        else:
            nc.vector.scalar_tensor_tensor(dst, pvc[:], 1.0, dst,
                                           ALU.mult, ALU.add)


def _emit_rep(nc, tc, x, y, rp, sp, sp1, ps_acc, ps_mm, ps_tr, dp,
              ident, ident_b, ones_r, one1, projw_r, bias_p, bias_pm1,
              bias_wr, bias_rd, G_b, wrd_r, kmT_f, vmT_f, qryT_r,
              load_wrd=None):
    AF = mybir.ActivationFunctionType
    ALU = mybir.AluOpType
    AX = mybir.AxisListType

    # key einsum partials accumulate in PSUM across phase 1; value
    # partials accumulate in SBUF (per-chunk PSUM tile + DVE add) to keep
    # the PSUM bank budget at 8
    pk_lo = ps_acc.tile([128, 512], F32, tag="slot_a")
    pk_hi = ps_acc.tile([128, 512], F32, tag="slot_b")
    pv_acc = rp.tile([128, 2 * 512], F32, tag="pv_acc")

    # ======================= PHASE 1 =====================================
    x_tiled = x.rearrange("(h t p) d -> h p t d", p=128, t=NBT)
    carry = None  # (kvT, vvT, expw_bm, rw) of the previous chunk
    for h in range(NCH):
        # ---- load + transpose x chunk -> xTr [128, dtile, CHUNK] f32r
        xTr = sp.tile([128, DT, CHUNK], F32R, tag="xTr")
        xa = sp.tile([128, NBT, D], F32, tag="xa", bufs=3)
        nc.sync.dma_start(xa[:], x_tiled[h])
        xTr_v = xTr.rearrange("p dt (t2 s j) -> p t2 s dt j", s=2, j=128)
        for half in range(2):
            ptx = ps_tr.tile([128, 2, DT, 128], F32, tag="tr")
            for s in range(2):
                t = 2 * half + s
                for dt in range(DT):
                    nc.tensor.matmul(ptx[:, s, dt, :],
                                     xa[:, t, dt * 128:(dt + 1) * 128],
                                     ident[:], is_transpose=True,
                                     start=True, stop=True,
                                     skip_group_check=True)
            nc.any.tensor_copy(xTr_v[:, half], ptx[:])

        # ---- projections + elu -> kvT/vvT (bf16), qryT (f32r)
        kvT = sp.tile([128, CHUNK], BF16, tag="kvT")
        vvT = sp.tile([128, CHUNK], BF16, tag="vvT")
        for j in range(3):
            pp = ps_mm.tile([128, CHUNK], F32, tag="mm")
            for dt in range(DT):
                nc.tensor.matmul(pp[:], projw_r[:, dt, j, :], xTr[:, dt, :],
                                 start=(dt == 0), stop=(dt == DT - 1))
            # elu(z+b) = [max(z+b-1, -1)] + [min(exp(z+b), 1)]
            # (exp is monotone, so exp(min(w,0)) = min(exp(w), 1))
            texp = sp.tile([128, CHUNK], F32, tag="texp", bufs=2)
            nc.scalar.activation(texp[:], pp[:], AF.Exp,
                                 bias=bias_p[:, j:j + 1])
            trelu = sp.tile([128, CHUNK], F32, tag="trelu", bufs=2)
            nc.vector.tensor_scalar(out=trelu[:], in0=pp[:],
                                    scalar1=bias_pm1[:, j:j + 1],
                                    scalar2=-1.0, op0=ALU.add, op1=ALU.max)
            dst = (kvT[:], vvT[:],
                   qryT_r[:, h * CHUNK:(h + 1) * CHUNK])[j]
            nc.vector.scalar_tensor_tensor(dst, texp[:], 1.0, trelu[:],
                                           ALU.min, ALU.add)

        # ---- write logits (via G) + exp + batched transpose to batch-major
        # expw_bm is mp-major [128, MT, NBT, 128] so each PSUM->SBUF copy is
        # contiguous; the einsum / reduce use strided views instead
        expw_bm = sp1.tile([128, MT, NBT, 128], BF16, tag="expw_bm", bufs=2)
        for mp in range(MT):
            pwl = ps_mm.tile([128, CHUNK], F32, tag="mm")
            nc.tensor.matmul(pwl[:], G_b[:, mp * 128:(mp + 1) * 128], kvT[:],
                             start=True, stop=True)
            eT = sp.tile([128, CHUNK], BF16, tag="eT", bufs=2)
            nc.scalar.activation(eT[:], pwl[:], AF.Exp,
                                 bias=bias_wr[:, mp:mp + 1])
            ptb = ps_tr.tile([128, NBT, 128], BF16, tag="trb", bufs=2)
            for t in range(NBT):
                nc.tensor.matmul(ptb[:, t, :], eT[:, t * 128:(t + 1) * 128],
                                 ident_b[:], is_transpose=True,
                                 start=True, stop=True, skip_group_check=True)
            nc.any.tensor_copy(expw_bm[:, mp], ptb[:])

        # ---- softmax denominators (per batch row)
        rw = sp.tile([128, NBT], F32, tag="rw")
        sw = sp.tile([128, NBT], F32, tag="sw")
        for t in range(NBT):
            nc.vector.tensor_reduce(sw[:, t:t + 1], expw_bm[:, :, t, :],
                                    AX.XY, ALU.add)
        nc.vector.reciprocal(rw[:], sw[:])

        # ---- einsum for the PREVIOUS chunk (skewed one chunk so PE has
        # this chunk's transposes/matmuls to chew on while DVE/ACT finish
        # the current chunk's exp/reduce chain)
        if carry is not None:
            _emit_einsum(nc, sp, ps_mm, ps_tr, ident_b, carry,
                         pk_lo, pk_hi, pv_acc,
                         first=(h == 1), last=False)
        carry = (kvT, vvT, expw_bm, rw)
    _emit_einsum(nc, sp, ps_mm, ps_tr, ident_b, carry,
                 pk_lo, pk_hi, pv_acc,
                 first=(NCH == 1), last=True)

    # ================== ALLREDUCE of partials ============================
    part_sb = rp.tile([128, M], F32, tag="part_sb")
    nc.vector.tensor_copy(part_sb[:, 0:512], pk_lo[:])
    nc.vector.tensor_copy(part_sb[:, 512:1024], pk_hi[:])
    cc_in = dp.tile([128, 2 * M], F32, tag="cc_in")
    cc_out = dp.tile([128, 2 * M], F32, tag="cc_out")
    nc.sync.dma_start(cc_in[:, 0:M], part_sb[:])
    nc.sync.dma_start(cc_in[:, M:2 * M], pv_acc[:])
    nc.gpsimd.collective_compute(
        "AllReduce", mybir.AluOpType.add,
        replica_groups=[list(range(N_CORES))],
        ins=[cc_in.opt()], outs=[cc_out.opt()],
    )
    red_sb = rp.tile([128, 2 * M], F32, tag="red_sb")
    nc.sync.dma_start(red_sb[:], cc_out[:])

    # ---- memory update + H = km_new.T @ Wrd (f32r) ----------------------
    if load_wrd is not None:
        load_wrd()
    km_newT = rp.tile([128, M], F32, tag="km_newT")
    nc.vector.scalar_tensor_tensor(km_newT[:], red_sb[:, 0:M], INV_B,
                                   kmT_f[:], ALU.mult, ALU.add)
    vm_newT = rp.tile([128, M], F32, tag="vm_newT")
    nc.vector.scalar_tensor_tensor(vm_newT[:], red_sb[:, M:2 * M], INV_B,
                                   vmT_f[:], ALU.mult, ALU.add)
    # transpose km_new/vm_new back to [m, *] blocks (f32r)
    kmn_mk = rp.tile([128, MT, 128], F32R, tag="kmn_mk")
    vmn_r = rp.tile([128, MT, 128], F32R, tag="vmn_r")
    for mk in range(MT):
        ptm = ps_tr.tile([128, 128], F32, tag="tr")
        nc.tensor.matmul(ptm[:], km_newT[:, mk * 128:(mk + 1) * 128],
                         ident[:], is_transpose=True, start=True, stop=True)
        nc.any.tensor_copy(kmn_mk[:, mk, :], ptm[:])
        ptm2 = ps_tr.tile([128, 128], F32, tag="tr")
        nc.tensor.matmul(ptm2[:], vm_newT[:, mk * 128:(mk + 1) * 128],
                         ident[:], is_transpose=True, start=True, stop=True)
        nc.any.tensor_copy(vmn_r[:, mk, :], ptm2[:])
    h_lo = ps_acc.tile([128, 512], F32, tag="slot_a")
    h_hi = ps_acc.tile([128, 512], F32, tag="slot_b")
    for mk in range(MT):
        nc.tensor.matmul(h_lo[:], kmn_mk[:, mk, :], wrd_r[mk][:, 0:512],
                         start=(mk == 0), stop=(mk == MT - 1),
                         skip_group_check=True)
        nc.tensor.matmul(h_hi[:], kmn_mk[:, mk, :], wrd_r[mk][:, 512:M],
                         start=(mk == 0), stop=(mk == MT - 1),
                         skip_group_check=True)
    H_r = rp.tile([128, M], F32R, tag="H_r")
    nc.vector.tensor_copy(H_r[:, 0:512], h_lo[:])
    nc.vector.tensor_copy(H_r[:, 512:M], h_hi[:])

    # ======================= PHASE 2 =====================================
    y_tiled = y.rearrange("(h t p) v -> h p t v", p=128, t=NBT)
    for h in range(NCH):
        qslice = qryT_r[:, h * CHUNK:(h + 1) * CHUNK]

        u_ps = ps_acc.tile([128, CHUNK], F32, tag="slot_a")
        s_ps = ps_acc.tile([1, CHUNK], F32, tag="slot_b")
        for mp in range(MT):
            prl = ps_mm.tile([128, CHUNK], F32, tag="mm")
            nc.tensor.matmul(prl[:], H_r[:, mp * 128:(mp + 1) * 128], qslice,
                             start=True, stop=True)
            erT = sp.tile([128, CHUNK], F32R, tag="erT", bufs=2)
            nc.scalar.activation(erT[:], prl[:], AF.Exp,
                                 bias=bias_rd[:, mp:mp + 1])
            nc.tensor.matmul(u_ps[:], vmn_r[:, mp, :], erT[:],
                             start=(mp == 0), stop=(mp == MT - 1),
                             skip_group_check=True)
            nc.tensor.matmul(s_ps[:], ones_r[:], erT[:],
                             start=(mp == 0), stop=(mp == MT - 1),
                             skip_group_check=True)

        # transpose denominators [1, CHUNK] -> [128, NBT] and invert
        s_sb = sp.tile([1, CHUNK], F32, tag="s_sb")
        nc.any.tensor_copy(s_sb[:], s_ps[:])
        s_cols = sp.tile([128, NBT], F32, tag="s_cols")
        for t in range(NBT):
            pst = ps_mm.tile([128, 1], F32, tag="mm")
            nc.tensor.matmul(pst[:], s_sb[0:1, t * 128:(t + 1) * 128],
                             one1[:], start=True, stop=True)
            nc.vector.tensor_copy(s_cols[:, t:t + 1], pst[:])
        r_cols = sp.tile([128, NBT], F32, tag="r_cols")
        nc.vector.reciprocal(r_cols[:], s_cols[:])

        # read_vec: transpose u back to batch-major, scale, store
        u_sb = sp.tile([128, CHUNK], F32, tag="u_sb")
        nc.any.tensor_copy(u_sb[:], u_ps[:])
        ot = sp.tile([128, NBT, V], F32, tag="ot", bufs=2)
        for t in range(NBT):
            ptu = ps_tr.tile([128, 128], F32, tag="tr")
            nc.tensor.matmul(ptu[:], u_sb[:, t * 128:(t + 1) * 128],
                             ident[:], is_transpose=True,
                             start=True, stop=True)
            nc.vector.tensor_scalar_mul(ot[:, t, :], ptu[:],
                                        r_cols[:, t:t + 1])
        nc.sync.dma_start(y_tiled[h], ot[:])


_NC_CACHE = None


def _get_nc():
    global _NC_CACHE
    if _NC_CACHE is None:
        _NC_CACHE = build_nc()
    return _NC_CACHE


def kernel(**inputs):
    nc = _get_nc()
    xs = np.ascontiguousarray(np.asarray(inputs["x"], dtype=np.float32))
    rep = {}
    for name in ("Wk", "Wv", "Wq", "Wwr", "Wrd", "key_memory", "value_memory"):
        rep[name] = np.ascontiguousarray(np.asarray(inputs[name], np.float32))
    for name in ("bk", "bv", "bq", "bwr", "brd"):
        rep[name] = np.ascontiguousarray(
            np.asarray(inputs[name], np.float32).reshape(-1, 1))
    in_maps = []
    for c in range(N_CORES):
        m = {"x": xs[c * B_LOC:(c + 1) * B_LOC]}
        m.update(rep)
        in_maps.append(m)
    res = run_bass_kernel_spmd(nc, in_maps, core_ids=list(range(N_CORES)))
    return np.concatenate([r["y"] for r in res.results], axis=0)
